# revision 1
# baseline (speedup 1.0000x reference)
"""Trainium2 Bass kernel for nn_CausalSelfAttention_73358041415963.

Math (literal reference semantics):
  Q/K/V = per-head projections of X;  S = Q @ K^T (no scale, no mask)
  A = softmax(S, axis=QUERY)  -> each key-column normalized over queries
  AV = A @ V;  literal reshape (B,H,N,DV)->(B,N,H*DV);  out = AV_r @ W_O

Key structural facts exploited:
  * softmax over the query axis i means A = E / colsum(E) with E = exp(S);
    AV = E @ (V / n[:,None]) where n[jk] = sum_i E[i, jk] -- normalization
    folds into V rows, no pass over the big E matrix.
  * the literal reshape maps head h to output rows n' in [h*128,(h+1)*128),
    so head-sharding needs NO collectives: each core owns 2 heads = 256
    output rows per batch.
  * TimelineSim matmul cost = out-free-size * cycles_per_row only, so the
    AV product uses E^T blocks as the STATIONARY operand (lhsT) and scaled
    V as the 64-wide moving operand: half the PE rows of the avp-oriented
    version.  bf16 keeps 1.0 cycles/row at free<256 (f32r would be 4.0).

Sharding: 8 cores x 2 heads. Each core gets full X, its 2 heads' W_Q/W_K/W_V
(packed [D,128]), full W_O. Core c returns output rows [256c, 256c+256).
"""

import numpy as np

import concourse.tile as tile
from concourse import bacc, mybir
from concourse.bass_utils import run_bass_kernel_spmd
from concourse.masks import make_identity

F32 = mybir.dt.float32
F32R = mybir.dt.float32r
BF16 = mybir.dt.bfloat16
P = 128
AF = mybir.ActivationFunctionType

# gpsimd software-DGE DMAs cast fp32 DRAM -> bf16 SBUF during the transfer.
CAST_DMA = True


def build_attn(tc, X, WQ, WK, WV, WO, O, N, D, DOUT):
    """Emit the per-core kernel into TileContext tc.

    X:  [2, N, D]    (full input, fp32)
    WQ/WK/WV: [D, 128]   2 local heads packed along the last axis
    WO: [16*64, DOUT]
    O:  [2, 2*(N//16), DOUT]   output rows for the 2 local heads
    """
    nc = tc.nc
    B, HL, SG = 2, 2, 16
    DCH = D // 128        # contraction chunks over model dim
    NCH = N // 512        # 512-wide chunks of sequence
    JKB = N // 128        # key blocks
    IHALF = N // 2        # scores processed in two i-halves
    CS = 512
    NCPH = IHALF // CS
    R = N // 16           # output rows per head

    with (
        tc.tile_pool(name="persist", bufs=1) as pp,
    ):
        ident = pp.tile([P, P], F32, tag="ident", name="ident")
        make_identity(nc, ident)
        identb = pp.tile([P, P], BF16, tag="identb", name="identb")
        nc.vector.tensor_copy(identb, ident)
        identr = pp.tile([P, P], F32R, tag="identr", name="identr")
        nc.vector.tensor_copy(identr, ident)
        # Dummy exp: forces the ACT Exp table load during the prologue
        # instead of at the first real score-exp.
        warm = pp.tile([P, 1], F32, tag="warm", name="warm")
        nc.scalar.activation(warm, ident[:, 0:1], AF.Exp)
        # zero rhs for the explicit PSUM-bank zeroing matmuls
        zb = pp.tile([P, 512], BF16, tag="zb", name="zb")
        nc.vector.memset(zb, 0.0)

        wq_sb = pp.tile([P, DCH, P], BF16, tag="wq", name="wq_sb")
        wk_sb = pp.tile([P, DCH, P], BF16, tag="wk", name="wk_sb")
        wv_sb = pp.tile([P, DCH, P], BF16, tag="wv", name="wv_sb")
        if CAST_DMA:
            nc.gpsimd.dma_start(wq_sb, WQ.rearrange("(dc p) m -> p dc m", p=P))
            nc.gpsimd.dma_start(wk_sb, WK.rearrange("(dc p) m -> p dc m", p=P))
            nc.gpsimd.dma_start(wv_sb, WV.rearrange("(dc p) m -> p dc m", p=P))
        else:
            wq_f = pp.tile([P, DCH, P], F32, tag="wqf", name="wq_f")
            wk_f = pp.tile([P, DCH, P], F32, tag="wkf", name="wk_f")
            wv_f = pp.tile([P, DCH, P], F32, tag="wvf", name="wv_f")
            nc.sync.dma_start(wq_f, WQ.rearrange("(dc p) m -> p dc m", p=P))
            nc.sync.dma_start(wk_f, WK.rearrange("(dc p) m -> p dc m", p=P))
            nc.sync.dma_start(wv_f, WV.rearrange("(dc p) m -> p dc m", p=P))
            nc.gpsimd.tensor_copy(wq_sb, wq_f)
            nc.gpsimd.tensor_copy(wk_sb, wk_f)
            nc.gpsimd.tensor_copy(wv_sb, wv_f)

        qT, kT, v_sb = [], [], []
        for b in range(B):
            qT.append(pp.tile([P, N], BF16, tag=f"qT{b}", name=f"qT{b}"))
            kT.append(pp.tile([P, N], BF16, tag=f"kT{b}", name=f"kT{b}"))
            v_sb.append(pp.tile([P, JKB, P], BF16, tag=f"v{b}", name=f"v{b}"))

        # Prefetch ALL of X up front as 8 wide DMAs (one per 512-row chunk).
        # Alternate between the software-DGE cast path (DMA_ENGINES) and the
        # HWDGE fp32 path + Pool convert: the two queues are independent
        # devices, so the two streams halve the X arrival time.
        xn4s = {}
        for b in range(B):
            for nch in range(NCH):
                n0 = nch * 512
                src = X[b, n0 : n0 + 512, :].rearrange("(ns p) d -> p ns d", p=P)
                xn4 = pp.tile([P, 4, D], BF16, tag="xn4", bufs=8, name="xn4")
                if b == 0 and nch == 0:
                    # split the first chunk so its first 128 rows land ~4x
                    # sooner — trims the cold-start PE gap.
                    for ns in range(4):
                        nc.gpsimd.dma_start(xn4[:, ns, :], src[:, ns, :])
                else:
                    nc.gpsimd.dma_start(xn4, src)
                xn4s[(b, nch)] = xn4

        # ---------------- Phase P: X^T, projections ----------------
        with (
            tc.tile_pool(name="pP", bufs=1) as sp,
            tc.tile_pool(name="pPps", bufs=1, space="PSUM") as pps,
        ):
            def emit_vtrans(pend):
                vt_pend, b_pend, nch_pend = pend
                tp2 = pps.tile([P, 512], BF16, tag="tp2", bufs=1, name="tp2")
                for ns in range(4):
                    nc.tensor.transpose(
                        tp2[:, ns * 128 : (ns + 1) * 128],
                        vt_pend[:, ns * 128 : (ns + 1) * 128],
                        identb,
                    )
                nc.vector.tensor_copy(
                    v_sb[b_pend][:, nch_pend * 4 : (nch_pend + 1) * 4, :], tp2
                )

            pending_vt = None
            for b in range(B):
                for nch in range(NCH):
                    xn4 = xn4s[(b, nch)]
                    xns = [xn4[:, ns, :] for ns in range(4)]
                    if pending_vt is not None:
                        emit_vtrans(pending_vt)
                        pending_vt = None
                    # Q^T, K^T, V^T for this n-chunk (2 heads packed on
                    # partitions). Per d-chunk: transpose X block, copy to
                    # SBUF, immediately run the 3 accumulating projections.
                    qt_ps = pps.tile([P, 512], F32, tag="qk", bufs=4, name="qt_ps")
                    kt_ps = pps.tile([P, 512], F32, tag="qk", bufs=4, name="kt_ps")
                    vt_ps = pps.tile([P, 512], F32, tag="qk", bufs=4, name="vt_ps")
                    # Software-pipelined by one d-chunk: the PE queue is
                    # in-order, so the projection matmuls for chunk dc are
                    # emitted after chunk dc+1's transposes — the PE streams
                    # transposes while the copy for dc drains on DVE.
                    def emit_mms(dc, xtc):
                        nc.tensor.matmul(
                            qt_ps, wq_sb[:, dc, :], xtc,
                            start=(dc == 0), stop=(dc == DCH - 1),
                        )
                        nc.tensor.matmul(
                            kt_ps, wk_sb[:, dc, :], xtc,
                            start=(dc == 0), stop=(dc == DCH - 1),
                        )
                        nc.tensor.matmul(
                            vt_ps, wv_sb[:, dc, :], xtc,
                            start=(dc == 0), stop=(dc == DCH - 1),
                        )

                    prev = None
                    for dc in range(DCH):
                        tp = pps.tile([P, 512], BF16, tag="tp", bufs=3, name="tp")
                        for ns in range(4):
                            nc.tensor.transpose(
                                tp[:, ns * 128 : (ns + 1) * 128],
                                xns[ns][:, dc * 128 : (dc + 1) * 128],
                                identb,
                            )
                        xtc = sp.tile([P, 512], BF16, tag="xtc", bufs=6, name="xtc")
                        # split the PSUM->SBUF drains across DVE and ACT: the
                        # ACT engine is otherwise idle during phase P, and DVE
                        # alone gates the tp-slot rotation.
                        if dc % 2 == 0:
                            nc.vector.tensor_copy(xtc, tp)
                        else:
                            nc.scalar.copy(xtc, tp)
                        if prev is not None:
                            emit_mms(dc - 1, prev)
                        prev = xtc
                    emit_mms(DCH - 1, prev)
                    nc.vector.tensor_copy(qT[b][:, nch * 512 : (nch + 1) * 512], qt_ps)
                    nc.scalar.copy(kT[b][:, nch * 512 : (nch + 1) * 512], kt_ps)
                    # V^T -> V natural via PE transposes, deferred to the
                    # start of the next chunk so the vt_sb drain never
                    # stalls the in-order PE queue.
                    vt_sb = sp.tile([P, 512], BF16, tag="vt", bufs=3, name="vt_sb")
                    nc.vector.tensor_copy(vt_sb, vt_ps)
                    pending_vt = (vt_sb, b, nch)

            if pending_vt is not None:
                emit_vtrans(pending_vt)
                pending_vt = None

        # W_O in natural row layout: chunk ic holds rows i in
        # [128*ic, 128*(ic+1)) on partitions — the W_O matmul contracts the
        # full 128-wide (s,dv) chunk at once (half the PE rows of the
        # 64-contraction variant) and needs no partition broadcast.
        OC = 256  # output column chunk
        wo2 = []
        for ic in range(DOUT // P):
            wo_t = pp.tile([P, DOUT], BF16, tag="wo", bufs=8, name="wo_t")
            nc.gpsimd.dma_start(wo_t, WO[ic * P : (ic + 1) * P, :])
            wo2.append(wo_t)

        # ---------------- Phase A: scores, exp, AV ----------------
        with (
            tc.tile_pool(name="pA", bufs=1) as ab,
            tc.tile_pool(name="pAps", bufs=1, space="PSUM") as aps,
        ):
            avct = {}

            def emit_wo_chunk(wb, idx):
                # Output projection chunk (one (dh, head) pair) for batch
                # wb; op tiles borrow "st"-tag PSUM slots.  Contracts the
                # full 128-wide (s,dv) chunks of AVc^T against natural-row
                # W_O chunks.
                dh, h = idx // HL, idx % HL
                act = avct[(wb, h)]
                opf = aps.tile([P, IHALF], F32, tag="st", bufs=2, name="opf")
                op = opf[:R, :OC]
                for ic in range(DOUT // P):
                    nc.tensor.matmul(
                        op,
                        act[:, ic, :],
                        wo2[ic][:, dh * OC : (dh + 1) * OC],
                        start=(ic == 0), stop=(ic == DOUT // P - 1),
                    )
                o_t = pp.tile([R, OC], F32, tag="ot", bufs=2, name="o_t")
                nc.vector.tensor_copy(o_t, op)
                nc.sync.dma_start(
                    O[wb, h * R : (h + 1) * R, dh * OC : (dh + 1) * OC], o_t
                )

            wo_pending = []
            for b in range(B):
                # AV^T accumulators: [queries-of-iblock, 64 dv] tiles packed
                # as avt[h][:, ib*64:(ib+1)*64], accumulated over all jk.
                avt = []
                for h in range(HL):
                    a = aps.tile([P, JKB * 64], F32, tag=f"avt{h}", bufs=1,
                                 name=f"avt{h}")
                    avt.append(a)
                    # Explicitly zero the banks with full-bank matmuls so the
                    # later 64-col accumulating writes (start=False) always
                    # land on zeroed PSUM regardless of lazy-zero semantics.
                    for bank in range(JKB * 64 // 512):
                        nc.tensor.matmul(
                            a[:, bank * 512 : (bank + 1) * 512],
                            identb, zb, start=True, stop=False,
                            skip_group_check=True,
                        )
                def emit_av(jk, es, nsum):
                    # normalization + AV^T for key-block jk: es blocks are
                    # the stationary lhsT (free dim 128 -> out partitions =
                    # queries), scaled V the 64-wide moving operand.
                    # Accumulates over jk into the pre-zeroed avt banks.
                    vsp = {}
                    for h in range(HL):
                        hs = slice(h * 64, (h + 1) * 64)
                        n1 = ab.tile([P, 1], F32, tag="n1", bufs=4, name="n1")
                        nc.vector.reduce_sum(n1, nsum[h], axis=mybir.AxisListType.X)
                        nr = ab.tile([P, 1], F32, tag="nr", bufs=4, name="nr")
                        nc.vector.reciprocal(nr, n1)
                        vs = ab.tile([P, 64], BF16, tag="vs", bufs=6, name="vs")
                        nc.vector.tensor_scalar_mul(vs, v_sb[b][:, jk, hs], nr)
                        vsp[h] = vs
                    for h in range(HL):
                        for half in range(2):
                            for cb in range(IHALF // 128):
                                ib = half * 8 + cb
                                nc.tensor.matmul(
                                    avt[h][:, ib * 64 : (ib + 1) * 64],
                                    es[(h, half)][:, cb * 128 : (cb + 1) * 128],
                                    vsp[h],
                                    start=False,
                                    stop=(jk == JKB - 1),
                                    skip_group_check=True,
                                )

                # Software pipeline: each jk's AV matmuls are emitted AFTER
                # the next jk's scores, so the in-order PE queue keeps the
                # score->exp chain (the ACT critical path) running ahead.
                pending_av = None
                for jk in range(JKB):
                    # interleave pending W_O chunks (previous batch) between
                    # jk iterations so they hide under this batch's exp time
                    # instead of stalling the ACT pipeline or trailing.
                    es = {}
                    nsum = {}
                    for h in range(HL):
                        nsum[h] = ab.tile([P, 2], F32, tag="nsum", bufs=4, name="nsum")
                    for half in range(2):
                        st = {}
                        for h in range(HL):
                            st[h] = aps.tile(
                                [P, IHALF], F32, tag="st", bufs=2, name="st"
                            )
                        for h in range(HL):
                            for c in range(NCPH):
                                hs = slice(h * 64, (h + 1) * 64)
                                i0 = half * IHALF + c * CS
                                nc.tensor.matmul(
                                    st[h][:, c * CS : (c + 1) * CS],
                                    kT[b][hs, jk * 128 : (jk + 1) * 128],
                                    qT[b][hs, i0 : i0 + CS],
                                    start=True, stop=True,
                                )
                        for h in range(HL):
                            e = ab.tile([P, IHALF], BF16, tag="e", bufs=12, name="e")
                            nc.scalar.activation(
                                e, st[h], AF.Exp,
                                accum_out=nsum[h][:, half : half + 1],
                            )
                            es[(h, half)] = e
                    if pending_av is not None:
                        emit_av(*pending_av)
                    elif wo_pending:
                        emit_wo_chunk(*wo_pending.pop(0))
                    if wo_pending and jk % 2 == 1:
                        emit_wo_chunk(*wo_pending.pop(0))
                    pending_av = (jk, es, nsum)
                emit_av(*pending_av)
                # Drain: avt (PSUM fp32, [q, dv] layout) -> bf16 SBUF ->
                # PE-transpose to AV^T-natural [dv, n] -> strided DVE copies
                # assemble AVc^T chunks ([s*64+dv partitions, r]) for W_O.
                # ps reuses the avt banks (free after the avts copy) so the
                # "st" exp-pipeline slots are never blocked by the drain.
                avts = {}
                for h in range(HL):
                    a_sb = ab.tile([P, JKB * 64], BF16, tag="avts", bufs=2,
                                   name="avts")
                    nc.vector.tensor_copy(a_sb, avt[h])
                    avts[h] = a_sb
                for h in range(HL):
                    ps = aps.tile([P, N], BF16, tag=f"avt{h}", bufs=1, name="ps")
                    for ib in range(JKB):
                        nc.tensor.transpose(
                            ps[0:64, ib * 128 : (ib + 1) * 128],
                            avts[h][:, ib * 64 : (ib + 1) * 64],
                            identb,
                        )
                    act = ab.tile([P, DOUT // P, P], BF16, tag="avct", bufs=4,
                                  name="avct")
                    psr = ps[0:64].rearrange("p (r s2 par) -> p par s2 r",
                                             par=2, s2=8)
                    nc.vector.tensor_copy(act[0:64], psr[:, 0])
                    nc.vector.tensor_copy(act[64:128], psr[:, 1])
                    avct[(b, h)] = act
                    if b == B - 1:
                        # tail: this head's W_O chunks right away, so they
                        # overlap the other head's drain.
                        for dh in range(DOUT // OC):
                            emit_wo_chunk(b, dh * HL + h)
                if b < B - 1:
                    wo_pending.extend(
                        (b, idx) for idx in range((DOUT // OC) * HL)
                    )
            for args in wo_pending:
                emit_wo_chunk(*args)


def build_nc(N=2048, D=1024, DOUT=1024, enable_asserts=False):
    """Build and compile the per-core Bass module. Returns nc."""
    nc = bacc.Bacc(
        "TRN2",
        target_bir_lowering=False,
        debug=False,
        enable_asserts=enable_asserts,
    )
    R = N // 16
    X = nc.dram_tensor("X", [2, N, D], F32R, kind="ExternalInput").ap()
    WQ = nc.dram_tensor("WQ", [D, 128], F32R, kind="ExternalInput").ap()
    WK = nc.dram_tensor("WK", [D, 128], F32R, kind="ExternalInput").ap()
    WV = nc.dram_tensor("WV", [D, 128], F32R, kind="ExternalInput").ap()
    WO = nc.dram_tensor("WO", [16 * 64, DOUT], F32, kind="ExternalInput").ap()
    O = nc.dram_tensor("O", [2, 2 * R, DOUT], F32, kind="ExternalOutput").ap()
    with tile.TileContext(nc) as tc:
        build_attn(tc, X, WQ, WK, WV, WO, O, N, D, DOUT)
    nc.compile()
    return nc


_NC_CACHE = {}


def _get_nc():
    if "full" not in _NC_CACHE:
        _NC_CACHE["full"] = build_nc()
    return _NC_CACHE["full"]


class _PjrtRunner:
    """Cached jitted SPMD executor (mirrors bass2jax.run_bass_via_pjrt but
    keeps the jitted callable so repeat calls skip re-trace/re-compile)."""

    def __init__(self, nc, n_cores=8):
        import jax
        from jax.experimental.shard_map import shard_map
        from jax.sharding import Mesh, PartitionSpec
        from concourse import bass2jax

        bass2jax.install_neuronx_cc_hook()
        self.n_cores = n_cores
        partition_name = (
            nc.partition_id_tensor.name if nc.partition_id_tensor else None
        )
        in_names, out_names, out_avals, zero_outs = [], [], [], []
        for alloc in nc.m.functions[0].allocations:
            if not isinstance(alloc, mybir.MemoryLocationSet):
                continue
            name = alloc.memorylocations[0].name
            if alloc.kind == "ExternalInput":
                if name != partition_name:
                    in_names.append(name)
            elif alloc.kind == "ExternalOutput":
                out_names.append(name)
                shape = tuple(alloc.tensor_shape)
                dtype = mybir.dt.np(alloc.dtype)
                out_avals.append(jax.core.ShapedArray(shape, dtype))
                zero_outs.append(np.zeros(shape, dtype))
        self.in_names = in_names
        self.out_names = out_names
        self.out_avals = out_avals
        self.zero_outs = zero_outs
        n_params = len(in_names)
        n_outs = len(out_names)
        all_names = list(in_names + out_names)
        if partition_name is not None:
            all_names.append(partition_name)
        all_names = tuple(all_names)

        def _body(*args):
            operands = list(args)
            if partition_name is not None:
                operands.append(bass2jax.partition_id_tensor())
            outs = bass2jax._bass_exec_p.bind(
                *operands,
                out_avals=tuple(out_avals),
                in_names=all_names,
                out_names=tuple(out_names),
                lowering_input_output_aliases=(),
                sim_require_finite=True,
                sim_require_nnan=True,
                nc=nc,
            )
            return tuple(outs)

        devices = jax.devices()[:n_cores]
        mesh = Mesh(np.asarray(devices), ("core",))
        donate = tuple(range(n_params, n_params + n_outs))
        self._fn = jax.jit(
            shard_map(
                _body,
                mesh=mesh,
                in_specs=(PartitionSpec("core"),) * (n_params + n_outs),
                out_specs=(PartitionSpec("core"),) * n_outs,
                check_rep=False,
            ),
            donate_argnums=donate,
            keep_unused=True,
        )

    def __call__(self, in_maps):
        import jax

        n = self.n_cores
        concat_in = [
            np.concatenate([np.asarray(m[nm]) for m in in_maps], axis=0)
            for nm in self.in_names
        ]
        concat_zeros = [
            np.zeros((n * z.shape[0], *z.shape[1:]), z.dtype) for z in self.zero_outs
        ]
        outs = self._fn(*concat_in, *concat_zeros)
        outs = [np.asarray(o) for o in jax.block_until_ready(outs)]
        return [
            {
                nm: outs[i].reshape(n, *self.out_avals[i].shape)[c]
                for i, nm in enumerate(self.out_names)
            }
            for c in range(n)
        ]


def _get_runner():
    if "runner" not in _NC_CACHE:
        _NC_CACHE["runner"] = _PjrtRunner(_get_nc())
    return _NC_CACHE["runner"]


def _make_in_maps(X, W_Q, W_K, W_V, W_O):
    X = np.ascontiguousarray(np.asarray(X), dtype=np.float32)
    W_Q = np.asarray(W_Q, dtype=np.float32)
    W_K = np.asarray(W_K, dtype=np.float32)
    W_V = np.asarray(W_V, dtype=np.float32)
    W_O = np.ascontiguousarray(np.asarray(W_O), dtype=np.float32)
    in_maps = []
    for c in range(8):
        wq = np.ascontiguousarray(
            np.concatenate([W_Q[2 * c], W_Q[2 * c + 1]], axis=1), dtype=np.float32
        )
        wk = np.ascontiguousarray(
            np.concatenate([W_K[2 * c], W_K[2 * c + 1]], axis=1), dtype=np.float32
        )
        wv = np.ascontiguousarray(
            np.concatenate([W_V[2 * c], W_V[2 * c + 1]], axis=1), dtype=np.float32
        )
        in_maps.append({"X": X, "WQ": wq, "WK": wk, "WV": wv, "WO": W_O})
    return in_maps


def kernel_with_results(X, W_Q, W_K, W_V, W_O, **run_kwargs):
    """Run via run_bass_kernel_spmd (supports trace kwargs); returns results."""
    nc = _get_nc()
    in_maps = _make_in_maps(X, W_Q, W_K, W_V, W_O)
    res = run_bass_kernel_spmd(nc, in_maps, core_ids=list(range(8)), **run_kwargs)
    return np.concatenate([r["O"] for r in res.results], axis=1), res


def kernel(X, W_Q, W_K, W_V, W_O):
    """Full-input entry point. X [2,2048,1024], W_Q/K/V [16,1024,64],
    W_O [1024,1024] -> [2,2048,1024] fp32."""
    try:
        runner = _get_runner()
        results = runner(_make_in_maps(X, W_Q, W_K, W_V, W_O))
        return np.concatenate([r["O"] for r in results], axis=1)
    except Exception:
        out, _ = kernel_with_results(X, W_Q, W_K, W_V, W_O)
        return out



# revision 27
# speedup vs baseline: 1.2734x; 1.2734x over previous
"""Trainium2 Bass kernel for nn_CausalSelfAttention_73358041415963.

Math (literal reference semantics):
  Q/K/V = per-head projections of X;  S = Q @ K^T (no scale, no mask)
  A = softmax(S, axis=QUERY)  -> each key-column normalized over queries
  AV = A @ V;  literal reshape (B,H,N,DV)->(B,N,H*DV);  out = AV_r @ W_O

Architecture (v1 -> v2 rewrite):
  * ACT is the hard bottleneck: 128 exp instructions of [128,1024]
    (~157us); v2 gives ACT *nothing else* and shapes the whole schedule
    so exp runs back-to-back from ~18us to the end.
  * Heads run sequentially per batch so PSUM fits:
    st(2x2 banks) + avt(2 banks) + aux(2 banks) = 8 banks.
  * Batch-0's X^T via PE transposes (fast lead-in); batch-1's X^T via
    gpsimd cast-DMA to a DRAM bf16 scratch + dma_start_transpose back
    (no PE/PSUM cost, hides under batch-0's ACT-bound windows).
  * K/V(b0), Q/K/V(b1) projections are "pump units" interleaved one per
    jk iteration inside the ACT-bound attention windows.
  * W_O of head h overlaps the next head's attention window via the aux
    PSUM slots; output rows per head are disjoint (literal-reshape
    structure) so no collectives.

Sharding: 8 cores x 2 heads. Each core gets full X, its 2 heads' W_Q/W_K/W_V
(packed [D,128]), full W_O. Core c returns output rows [256c, 256c+256).
"""

import numpy as np

import concourse.tile as tile
from concourse import bacc, mybir
from concourse.bass_utils import run_bass_kernel_spmd
from concourse.masks import make_identity

F32 = mybir.dt.float32
F32R = mybir.dt.float32r
BF16 = mybir.dt.bfloat16
P = 128
AF = mybir.ActivationFunctionType


def build_attn(tc, X, WQKV, WO, XB, O, N, D, DOUT):
    """Emit the per-core kernel into TileContext tc.

    X:  [2, N, D]    (full input, fp32 DRAM)
    WQKV: [D, 384]   W_Q|W_K|W_V, each with 2 local heads packed ([D,128])
    WO: [16*64, DOUT]
    XB: [2, N, D] bf16 DRAM scratch (DMA-transpose roundtrip: b0 rows
        1024:, all of b1)
    O:  [2, 2*(N//16), DOUT]   output rows for the 2 local heads
    """
    nc = tc.nc
    B, HL = 2, 2
    DCH = D // 128        # 8 contraction chunks over model dim
    NCH = N // 512        # 4 sequence chunks
    JKB = N // 128        # 16 key blocks
    HF = N // 2           # 1024 exp tile width (i-half)
    R = N // 16           # 128 output rows per head
    OC = 256              # W_O output column chunk

    with (
        tc.tile_pool(name="persist", bufs=1) as pp,
        tc.tile_pool(name="ppsA", bufs=1, space="PSUM") as aps,
    ):
        ident = pp.tile([P, P], F32, tag="ident", name="ident")
        make_identity(nc, ident)
        identb = pp.tile([P, P], BF16, tag="identb", name="identb")
        nc.vector.tensor_copy(identb, ident)
        # Dummy exp: loads the ACT Exp table during the prologue.
        warm = pp.tile([P, 1], F32, tag="warm", name="warm")
        nc.scalar.activation(warm, ident[:, 0:1], AF.Exp)
        # zero rhs for explicit PSUM-bank zeroing matmuls
        zb = pp.tile([P, 512], BF16, tag="zb", name="zb")
        nc.vector.memset(zb, 0.0)

        # ---------- DMA prologue (order == service order; DMA dev ~serial)
        # Critical chain: X-c0 -> wqkv -> X-c1 feed the warmup sweep; b0's
        # c2/c3 go via the bf16-DRAM roundtrip + DMA transpose (no PE cost),
        # landing just in time for the half1 sweep.
        wqkv_sb = pp.tile([P, DCH, 384], BF16, tag="wqkv", name="wqkv_sb")
        wq_sb = wqkv_sb[:, :, 0:128]
        wk_sb = wqkv_sb[:, :, 128:256]
        wv_sb = wqkv_sb[:, :, 256:384]
        xb0 = []
        for c in range(2):
            t = pp.tile([P, 4, D], BF16, tag="xb0", bufs=2, name=f"xb0_{c}")
            src = X[0, c * 512 : (c + 1) * 512, :].rearrange(
                "(ns p) d -> p ns d", p=P
            )
            nc.gpsimd.dma_start(t, src)
            if c == 0:
                nc.gpsimd.dma_start(
                    wqkv_sb, WQKV.rearrange("(dc p) m -> p dc m", p=P)
                )
            xb0.append(t)
        # b0 c2/c3: cast to DRAM bf16 + DMA-transpose back.  Everything on
        # the serialized DMA device after this point is explicitly chained
        # (chain_iter_dep) behind the lead-critical xt0 transposes.
        xt0 = {}
        for c in (2, 3):
            nc.gpsimd.dma_start(
                XB[0, c * 512 : (c + 1) * 512, :], X[0, c * 512 : (c + 1) * 512, :]
            )
        for c in (2, 3):
            t = pp.tile([P, DCH, 512], BF16, tag="xt1", bufs=6, name=f"xt0_{c}")
            ti = nc.sync.dma_start_transpose(t, XB[0, c * 512 : (c + 1) * 512, :])
            tc.chain_iter_dep("dmaorder", ti.ins)
            xt0[c] = t
        # W_O natural rows: wo_sb[:, ic, :] holds rows [128*ic, 128*(ic+1))
        wo_sb = pp.tile([P, DCH, DOUT], BF16, tag="wo", name="wo_sb")
        xt1 = []
        for c in range(NCH):
            t = pp.tile([P, DCH, 512], BF16, tag="xt1", bufs=6, name=f"xt1_{c}")
            xt1.append(t)

        def emit_late_dmas():
            """The framework's (whole-tensor) XB dependency tracking orders
            these casts after the lead-critical xt0 DMA-transposes; wo goes
            last so its 5.8us never splits the xt0 chain on the serialized
            DMA device."""
            for c in range(NCH):
                ci = nc.gpsimd.dma_start(
                    XB[1, c * 512 : (c + 1) * 512, :],
                    X[1, c * 512 : (c + 1) * 512, :],
                )
                tc.chain_iter_dep("dmaorder", ci.ins)
                ti = nc.sync.dma_start_transpose(
                    xt1[c], XB[1, c * 512 : (c + 1) * 512, :]
                )
                tc.chain_iter_dep("dmaorder", ti.ins)
            wi = nc.gpsimd.dma_start(wo_sb, WO.rearrange("(ic p) o -> p ic o", p=P))
            tc.chain_iter_dep("dmaorder", wi.ins)

        qT, kT, v_sb = [], [], []
        for b in range(B):
            qT.append(pp.tile([P, N], BF16, tag=f"qT{b}", name=f"qT{b}"))
            kT.append(pp.tile([P, N], BF16, tag=f"kT{b}", name=f"kT{b}"))
            v_sb.append(pp.tile([P, JKB, P], BF16, tag=f"v{b}", name=f"v{b}"))

        # xtc: b0's X^T chunks (PE-transposed), kept for K/V(b0) pump units;
        # grouped 4 dc per tile: xtc[(c, q)][:, dc % 4, :]
        xtc2 = {}  # (c, dc//4) -> [P, 4, 512] bf16

        def xtc(c, dc):
            return xtc2[(c, dc // 4)][:, dc % 4, :]

        # avt PSUM accumulators AV^T[h]: [i-queries, (ib, dv)] - 2 banks
        # aux PSUM: proj accumulators / W_O op tiles - 2x1 banks

        def emit_qk_proj(b, which, c, xsrc, width=512):
            """Projection chunk: qT/kT[b][:, c*width + [0,width)] over 8 dc.
            width=256 halves the PE burst for pump units."""
            w = wq_sb if which == "q" else wk_sb
            dst = qT[b] if which == "q" else kT[b]
            ps = aps.tile([P, 512], F32, tag="aux", bufs=2, name=f"p{which}{b}{c}")
            pw = ps[:, 0:width]
            for dc in range(DCH):
                nc.tensor.matmul(
                    pw, w[:, dc, :], xsrc(dc)[:, 0:width],
                    start=(dc == 0), stop=(dc == DCH - 1),
                )
            nc.vector.tensor_copy(dst[:, c * width : c * width + width], pw)

        def emit_v_proj(b, c2, xsrc):
            """V natural rows for TWO key-blocks (quarter-chunk c2): out
            [n-128-block, 128 dv] with x^T block as stationary. The range is
            zeroed with one wide matmul first so the two column-disjoint
            accumulation groups can use start=False (PSUM lazy-zero marks
            whole regions; per-group start=True would clobber siblings)."""
            ps = aps.tile([P, 512], F32, tag="aux", bufs=2, name=f"pv{b}{c2}")
            pw = ps[:, 0:256]
            nc.tensor.matmul(pw, identb, zb[:, 0:256], start=True, stop=False,
                             skip_group_check=True)
            for dc in range(DCH):
                for jb in range(2):
                    nc.tensor.matmul(
                        pw[:, jb * P : (jb + 1) * P],
                        xsrc(dc)[:, (c2 % 2) * 256 + jb * P : (c2 % 2) * 256 + (jb + 1) * P],
                        wv_sb[:, dc, :],
                        start=False, stop=(dc == DCH - 1),
                        skip_group_check=True,
                    )
            nc.vector.tensor_copy(
                v_sb[b][:, c2 * 2 : (c2 + 1) * 2, :], pw
            )

        # ---------------- lead-in: b0 c0/c1 transposes + Q + K(c0,c1)
        # tp borrows "st" slots (4KB, idle until the first window).
        for c in range(2):
            xns = [xb0[c][:, ns, :] for ns in range(4)]
            for q in range(2):
                tp = aps.tile([P, 4, 512], BF16, tag="st", bufs=2, name="tp")
                for dl in range(4):
                    dc = q * 4 + dl
                    for ns in range(4):
                        nc.tensor.transpose(
                            tp[:, dl, ns * 128 : (ns + 1) * 128],
                            xns[ns][:, dc * 128 : (dc + 1) * 128],
                            identb,
                        )
                xc = pp.tile([P, 4, 512], BF16, tag="xtc", bufs=2 * NCH,
                             name=f"xtc{c}_{q}")
                nc.vector.tensor_copy(xc, tp)
                xtc2[(c, q)] = xc
            emit_qk_proj(0, "q", c, lambda dc, c=c: xtc(c, dc))
        emit_qk_proj(0, "k", 0, lambda dc: xtc(0, dc))
        emit_qk_proj(0, "k", 1, lambda dc: xtc(1, dc))

        # ---------------- pump units (interleaved into jk loops)
        # Order matters: a unit writing a tile must be EMITTED before any
        # reader (the tile framework only syncs backward).  V(b0)c0 first
        # (AV(jk0) reads it at jk1), then K/V alternating so kT chunk c is
        # emitted before scores jk>=4c and v chunk c before AV jk>=4c.
        # all units are ~0.9us of PE: 256-wide proj columns / 2 V-blocks.
        # xsrcq(b, c2) yields the 256-wide moving slice for quarter c2.
        def xs0(c2):
            if c2 < 4:
                return lambda dc, c2=c2: xtc(c2 // 2, dc)[:, (c2 % 2) * 256 : (c2 % 2 + 1) * 256]
            return lambda dc, c2=c2: xt0[c2 // 2][:, dc, (c2 % 2) * 256 : (c2 % 2 + 1) * 256]

        def xs1(c2):
            return lambda dc, c2=c2: xt1[c2 // 2][:, dc, (c2 % 2) * 256 : (c2 % 2 + 1) * 256]

        def xv0(c2):
            if c2 < 4:
                return lambda dc, c2=c2: xtc(c2 // 2, dc)
            return lambda dc, c2=c2: xt0[c2 // 2][:, dc, :]

        def xv1(c2):
            return lambda dc, c2=c2: xt1[c2 // 2][:, dc, :]

        # (deadline_pos, fn): deadline = last pump position (w*16+jk) at
        # which the unit must be EMITTED so every reader is emitted later.
        units = []
        for c2 in range(2 * NCH):
            # reader: AV(jk=2*c2) emitted at pos 2*c2+2
            units.append((2 * c2 + 1,
                          lambda c2=c2: emit_v_proj(0, c2, xv0(c2))))
        for c2 in range(4, 2 * NCH):
            # reader: sc(jk=2*c2) at pos 2*c2
            units.append((2 * c2 - 2,
                          lambda c2=c2: emit_qk_proj(0, "k", c2, xs0(c2), 256)))
        for c2 in range(2 * NCH):
            # reader: sc(W3, jk0) at pos 32
            units.append((14 + c2,
                          lambda c2=c2: emit_qk_proj(1, "q", c2, xs1(c2), 256)))
        kdl = [22, 24, 26, 28, 32, 34, 36, 38]
        for c2 in range(2 * NCH):
            # reader: sc(W3, jk=2*c2) at pos 32+2*c2
            units.append((kdl[c2],
                          lambda c2=c2: emit_qk_proj(1, "k", c2, xs1(c2), 256)))
        for c2 in range(2 * NCH):
            # reader: AV(W3, jk=2*c2) at pos 34+2*c2
            units.append((33 + 2 * c2,
                          lambda c2=c2: emit_v_proj(1, c2, xv1(c2))))
        units.sort(key=lambda t: t[0])
        front = []       # drain/WO pieces, pumped in the slack
        pos_ref = [0]    # current pump position

        def force_due():
            # Emission-order correctness: every unit whose reader gets
            # emitted this iteration MUST be emitted first.
            while units and units[0][0] <= pos_ref[0]:
                units.pop(0)[1]()

        def pump(n=1):
            # Non-greedy: proj units only near their deadline, drain/W_O
            # pieces fill the slack, otherwise leave the PE light -- the
            # global PE budget fits but bunching starves the exp stream.
            for _ in range(n):
                if units and units[0][0] <= pos_ref[0] + 1:
                    units.pop(0)[1]()
                elif front and pos_ref[0] % JKB >= 2:
                    front.pop(0)()

        # ---------------- W_O chunk for a finished head
        def emit_wo_chunk(b, h, act, dh):
            opf = aps.tile([P, 512], F32, tag="aux", bufs=2, name="opf")
            op = opf[:R, :OC]
            for ic in range(DCH):
                nc.tensor.matmul(
                    op, act[:, ic, :],
                    wo_sb[:, ic, dh * OC : (dh + 1) * OC],
                    start=(ic == 0), stop=(ic == DCH - 1),
                )
            o_t = pp.tile([R, OC], F32, tag="ot", bufs=3, name="o_t")
            nc.vector.tensor_copy(o_t, op)
            nc.sync.dma_start(
                O[b, h * R : (h + 1) * R, dh * OC : (dh + 1) * OC], o_t
            )

        def emit_drain_pieces(b, h, a_sb, via_front):
            """PE-transpose avts blocks to AV^T-natural, assemble AVc^T
            ([s*64+dv partitions, r]) chunks, then the 4 W_O chunks.
            The transpose targets borrow aux slots (two 2KB halves)."""
            act = pp.tile([P, DCH, P], BF16, tag="avct", bufs=2, name="avct")

            def transposes(half):
                ps = aps.tile([P, HF], BF16, tag="aux", bufs=2, name="psT")
                for cb in range(8):
                    ib = half * 8 + cb
                    nc.tensor.transpose(
                        ps[0:64, cb * 128 : (cb + 1) * 128],
                        a_sb[:, ib * 64 : (ib + 1) * 64],
                        identb,
                    )
                psr = ps[0:64].rearrange(
                    "p (r s2 par) -> p par s2 r", par=2, s2=8
                )
                rs = slice(half * 64, (half + 1) * 64)
                nc.vector.tensor_copy(act[0:64, :, rs], psr[:, 0])
                nc.vector.tensor_copy(act[64:128, :, rs], psr[:, 1])

            pieces = [lambda: transposes(0), lambda: transposes(1)] + [
                (lambda dh=dh: emit_wo_chunk(b, h, act, dh))
                for dh in range(DOUT // OC)
            ]
            if via_front:
                front.extend(pieces)
            else:
                for p in pieces:
                    p()

        # ---------------- attention: sequential heads, pipelined jk loop.
        # AV(jk) is emitted TWO jk steps late: its dependency chain
        # (exp accum -> DVE nsum/recip/scale -> 16 matmuls) resolves ~1.5us
        # after exp(jk) ends; at lag 1 it head-of-line-blocks the next
        # scores in the in-order PE queue and stretches every jk period.
        pending_av = []  # (b, hs, avt, jk, es0, es1, nsum)

        def emit_av(b, hs, avt, jk, es0, es1, nsum):
            n1 = pp.tile([P, 1], F32, tag="n1", bufs=6, name="n1")
            nc.vector.reduce_sum(n1, nsum, axis=mybir.AxisListType.X)
            nr = pp.tile([P, 1], F32, tag="nr", bufs=6, name="nr")
            nc.vector.reciprocal(nr, n1)
            vs = pp.tile([P, 64], BF16, tag="vs", bufs=6, name="vs")
            nc.vector.tensor_scalar_mul(vs, v_sb[b][:, jk, hs], nr)
            for half, es in ((0, es0), (1, es1)):
                for cb in range(HF // 128):
                    ib = half * 8 + cb
                    nc.tensor.matmul(
                        avt[:, ib * 64 : (ib + 1) * 64],
                        es[:, cb * 128 : (cb + 1) * 128],
                        vs,
                        start=False, stop=(jk == JKB - 1),
                        skip_group_check=True,
                    )

        def pop_av(n=1, min_len=2):
            # Only emit entries that are >= 2 jk steps old (lag-2 pipeline);
            # the tail flushes with min_len=1.
            for _ in range(n):
                if len(pending_av) >= min_len:
                    emit_av(*pending_av.pop(0))

        def emit_half(b, hs, jk, half, nsum):
            st = aps.tile([P, HF], F32, tag="st", bufs=2, name="st")
            i0 = half * HF
            for cq in range(2):
                nc.tensor.matmul(
                    st[:, cq * 512 : (cq + 1) * 512],
                    kT[b][hs, jk * 128 : (jk + 1) * 128],
                    qT[b][hs, i0 + cq * 512 : i0 + (cq + 1) * 512],
                    start=True, stop=True,
                )
            e = pp.tile([P, HF], BF16, tag="e", bufs=14, name="e")
            nc.scalar.activation(
                e, st, AF.Exp, accum_out=nsum[:, half : half + 1]
            )
            return e

        def attention_window(b, h, prev, widx):
            hs = slice(h * 64, (h + 1) * 64)
            avt = None
            jk_start = 0
            if widx == 0:
                # Warmup: half0 of jk0-7 only needs qT chunks 0-1 and kT
                # chunks 0-1 (lead-in) -- exp starts ~15us while the c2/c3
                # DMA-transposes + Q projections finish underneath.
                warm_es0, warm_nsum = [], []
                for jk in range(5):
                    nsum = pp.tile([P, 2], F32, tag="nsum", bufs=10, name="nsum")
                    warm_nsum.append(nsum)
                    warm_es0.append(emit_half(b, hs, jk, 0, nsum))
                emit_qk_proj(0, "q", 2, lambda dc: xt0[2][:, dc, :])
                emit_qk_proj(0, "q", 3, lambda dc: xt0[3][:, dc, :])
                emit_late_dmas()
                avt = aps.tile([P, JKB * 64], F32, tag="avt", bufs=1,
                               name="avt00")
                for bank in range(JKB * 64 // 512):
                    nc.tensor.matmul(
                        avt[:, bank * 512 : (bank + 1) * 512],
                        identb, zb, start=True, stop=False,
                        skip_group_check=True,
                    )
                for jk in range(5):
                    pos_ref[0] = jk
                    e1 = emit_half(b, hs, jk, 1, warm_nsum[jk])
                    force_due()
                    pop_av(1)
                    pump(1)
                    pending_av.append(
                        (b, hs, avt, jk, warm_es0[jk], e1, warm_nsum[jk])
                    )
                jk_start = 5
            for jk in range(jk_start, JKB):
                pos_ref[0] = widx * JKB + jk
                es = []
                nsum = pp.tile([P, 2], F32, tag="nsum", bufs=10, name="nsum")
                for half in range(2):
                    es.append(emit_half(b, hs, jk, half, nsum))
                if jk == 2:
                    # The previous window's last pending AV was emitted at
                    # jk1; its avt is final: drain it, free the slot,
                    # allocate + zero ours (before popping this window's
                    # own jk0 entry), queue its layout/W_O pieces.
                    if prev is not None:
                        pb, ph, pavt = prev
                        a_sb = pp.tile([P, JKB * 64], BF16, tag="avts",
                                       bufs=2, name="avts")
                        nc.vector.tensor_copy(a_sb, pavt)
                    avt = aps.tile([P, JKB * 64], F32, tag="avt", bufs=1,
                                   name=f"avt{b}{h}")
                    for bank in range(JKB * 64 // 512):
                        nc.tensor.matmul(
                            avt[:, bank * 512 : (bank + 1) * 512],
                            identb, zb, start=True, stop=False,
                            skip_group_check=True,
                        )
                    for i, ent in enumerate(pending_av):
                        if ent[2] is None:
                            pending_av[i] = ent[:2] + (avt,) + ent[3:]
                    force_due()
                    pop_av(1)
                    if prev is not None:
                        emit_drain_pieces(pb, ph, a_sb, via_front=True)
                else:
                    force_due()
                    pop_av(1)
                pump(1)
                pending_av.append((b, hs, avt, jk, es[0], es[1], nsum))
            return avt

        prev = None  # (b, h, avt)
        for b in range(B):
            for h in range(HL):
                avt = attention_window(b, h, prev, b * HL + h)
                prev = (b, h, avt)
        # tail: flush remaining AVs, then the last head's drain + W_O
        pop_av(2, min_len=1)
        pb, ph, pavt = prev
        a_sb = pp.tile([P, JKB * 64], BF16, tag="avts", bufs=2, name="avts")
        nc.vector.tensor_copy(a_sb, pavt)
        emit_drain_pieces(pb, ph, a_sb, via_front=False)


def build_nc(N=2048, D=1024, DOUT=1024, enable_asserts=False):
    """Build and compile the per-core Bass module. Returns nc."""
    nc = bacc.Bacc(
        "TRN2",
        target_bir_lowering=False,
        debug=False,
        enable_asserts=enable_asserts,
    )
    R = N // 16
    X = nc.dram_tensor("X", [2, N, D], F32R, kind="ExternalInput").ap()
    WQKV = nc.dram_tensor("WQKV", [D, 384], F32R, kind="ExternalInput").ap()
    WO = nc.dram_tensor("WO", [16 * 64, DOUT], F32, kind="ExternalInput").ap()
    XB = nc.dram_tensor("XB", [2, N, D], BF16, kind="Internal").ap()
    O = nc.dram_tensor("O", [2, 2 * R, DOUT], F32, kind="ExternalOutput").ap()
    with tile.TileContext(nc) as tc:
        build_attn(tc, X, WQKV, WO, XB, O, N, D, DOUT)
    nc.compile()
    return nc


_NC_CACHE = {}


def _get_nc():
    if "full" not in _NC_CACHE:
        _NC_CACHE["full"] = build_nc()
    return _NC_CACHE["full"]


class _PjrtRunner:
    """Cached jitted SPMD executor (mirrors bass2jax.run_bass_via_pjrt but
    keeps the jitted callable so repeat calls skip re-trace/re-compile)."""

    def __init__(self, nc, n_cores=8):
        import jax
        from jax.experimental.shard_map import shard_map
        from jax.sharding import Mesh, PartitionSpec
        from concourse import bass2jax

        bass2jax.install_neuronx_cc_hook()
        self.n_cores = n_cores
        partition_name = (
            nc.partition_id_tensor.name if nc.partition_id_tensor else None
        )
        in_names, out_names, out_avals, zero_outs = [], [], [], []
        for alloc in nc.m.functions[0].allocations:
            if not isinstance(alloc, mybir.MemoryLocationSet):
                continue
            name = alloc.memorylocations[0].name
            if alloc.kind == "ExternalInput":
                if name != partition_name:
                    in_names.append(name)
            elif alloc.kind == "ExternalOutput":
                out_names.append(name)
                shape = tuple(alloc.tensor_shape)
                dtype = mybir.dt.np(alloc.dtype)
                out_avals.append(jax.core.ShapedArray(shape, dtype))
                zero_outs.append(np.zeros(shape, dtype))
        self.in_names = in_names
        self.out_names = out_names
        self.out_avals = out_avals
        self.zero_outs = zero_outs
        n_params = len(in_names)
        n_outs = len(out_names)
        all_names = list(in_names + out_names)
        if partition_name is not None:
            all_names.append(partition_name)
        all_names = tuple(all_names)

        def _body(*args):
            operands = list(args)
            if partition_name is not None:
                operands.append(bass2jax.partition_id_tensor())
            outs = bass2jax._bass_exec_p.bind(
                *operands,
                out_avals=tuple(out_avals),
                in_names=all_names,
                out_names=tuple(out_names),
                lowering_input_output_aliases=(),
                sim_require_finite=True,
                sim_require_nnan=True,
                nc=nc,
            )
            return tuple(outs)

        devices = jax.devices()[:n_cores]
        mesh = Mesh(np.asarray(devices), ("core",))
        donate = tuple(range(n_params, n_params + n_outs))
        self._fn = jax.jit(
            shard_map(
                _body,
                mesh=mesh,
                in_specs=(PartitionSpec("core"),) * (n_params + n_outs),
                out_specs=(PartitionSpec("core"),) * n_outs,
                check_rep=False,
            ),
            donate_argnums=donate,
            keep_unused=True,
        )

    def __call__(self, in_maps):
        import jax

        n = self.n_cores
        concat_in = [
            np.concatenate([np.asarray(m[nm]) for m in in_maps], axis=0)
            for nm in self.in_names
        ]
        concat_zeros = [
            np.zeros((n * z.shape[0], *z.shape[1:]), z.dtype) for z in self.zero_outs
        ]
        outs = self._fn(*concat_in, *concat_zeros)
        outs = [np.asarray(o) for o in jax.block_until_ready(outs)]
        return [
            {
                nm: outs[i].reshape(n, *self.out_avals[i].shape)[c]
                for i, nm in enumerate(self.out_names)
            }
            for c in range(n)
        ]


def _get_runner():
    if "runner" not in _NC_CACHE:
        _NC_CACHE["runner"] = _PjrtRunner(_get_nc())
    return _NC_CACHE["runner"]


def _make_in_maps(X, W_Q, W_K, W_V, W_O):
    X = np.ascontiguousarray(np.asarray(X), dtype=np.float32)
    W_Q = np.asarray(W_Q, dtype=np.float32)
    W_K = np.asarray(W_K, dtype=np.float32)
    W_V = np.asarray(W_V, dtype=np.float32)
    W_O = np.ascontiguousarray(np.asarray(W_O), dtype=np.float32)
    in_maps = []
    for c in range(8):
        wqkv = np.ascontiguousarray(
            np.concatenate(
                [W_Q[2 * c], W_Q[2 * c + 1], W_K[2 * c], W_K[2 * c + 1],
                 W_V[2 * c], W_V[2 * c + 1]], axis=1
            ),
            dtype=np.float32,
        )
        in_maps.append({"X": X, "WQKV": wqkv, "WO": W_O})
    return in_maps


def kernel_with_results(X, W_Q, W_K, W_V, W_O, **run_kwargs):
    """Run via run_bass_kernel_spmd (supports trace kwargs); returns results."""
    nc = _get_nc()
    in_maps = _make_in_maps(X, W_Q, W_K, W_V, W_O)
    res = run_bass_kernel_spmd(nc, in_maps, core_ids=list(range(8)), **run_kwargs)
    return np.concatenate([r["O"] for r in res.results], axis=1), res


def kernel(X, W_Q, W_K, W_V, W_O):
    """Full-input entry point. X [2,2048,1024], W_Q/K/V [16,1024,64],
    W_O [1024,1024] -> [2,2048,1024] fp32."""
    try:
        runner = _get_runner()
        results = runner(_make_in_maps(X, W_Q, W_K, W_V, W_O))
        return np.concatenate([r["O"] for r in results], axis=1)
    except Exception:
        out, _ = kernel_with_results(X, W_Q, W_K, W_V, W_O)
        return out


# revision 35
# speedup vs baseline: 1.3034x; 1.0236x over previous
"""Trainium2 Bass kernel for nn_CausalSelfAttention_73358041415963.

Math (literal reference semantics):
  Q/K/V = per-head projections of X;  S = Q @ K^T (no scale, no mask)
  A = softmax(S, axis=QUERY)  -> each key-column normalized over queries
  AV = A @ V;  literal reshape (B,H,N,DV)->(B,N,H*DV);  out = AV_r @ W_O

Architecture (v1 -> v2 rewrite):
  * ACT is the hard bottleneck: 128 exp instructions of [128,1024]
    (~157us); v2 gives ACT *nothing else* and shapes the whole schedule
    so exp runs back-to-back from ~18us to the end.
  * Heads run sequentially per batch so PSUM fits:
    st(2x2 banks) + avt(2 banks) + aux(2 banks) = 8 banks.
  * Batch-0's X^T via PE transposes (fast lead-in); batch-1's X^T via
    gpsimd cast-DMA to a DRAM bf16 scratch + dma_start_transpose back
    (no PE/PSUM cost, hides under batch-0's ACT-bound windows).
  * K/V(b0), Q/K/V(b1) projections are "pump units" interleaved one per
    jk iteration inside the ACT-bound attention windows.
  * W_O of head h overlaps the next head's attention window via the aux
    PSUM slots; output rows per head are disjoint (literal-reshape
    structure) so no collectives.

Sharding: 8 cores x 2 heads. Each core gets full X, its 2 heads' W_Q/W_K/W_V
(packed [D,128]), full W_O. Core c returns output rows [256c, 256c+256).
"""

import numpy as np

import concourse.tile as tile
from concourse import bacc, mybir
from concourse.bass_utils import run_bass_kernel_spmd
from concourse.masks import make_identity

F32 = mybir.dt.float32
F32R = mybir.dt.float32r
BF16 = mybir.dt.bfloat16
P = 128
AF = mybir.ActivationFunctionType


def build_attn(tc, X, WQKV, WO, XB, O, N, D, DOUT):
    """Emit the per-core kernel into TileContext tc.

    X:  [2, N, D]    (full input, fp32 DRAM)
    WQKV: [D, 384]   W_Q|W_K|W_V, each with 2 local heads packed ([D,128])
    WO: [16*64, DOUT]
    XB: [2, N, D] bf16 DRAM scratch (DMA-transpose roundtrip: b0 rows
        1024:, all of b1)
    O:  [2, 2*(N//16), DOUT]   output rows for the 2 local heads
    """
    nc = tc.nc
    B, HL = 2, 2
    DCH = D // 128        # 8 contraction chunks over model dim
    NCH = N // 512        # 4 sequence chunks
    JKB = N // 128        # 16 key blocks
    HF = N // 2           # 1024 exp tile width (i-half)
    R = N // 16           # 128 output rows per head
    OC = 256              # W_O output column chunk

    with (
        tc.tile_pool(name="persist", bufs=1) as pp,
        tc.tile_pool(name="ppsA", bufs=1, space="PSUM") as aps,
    ):
        ident = pp.tile([P, P], F32, tag="ident", name="ident")
        make_identity(nc, ident)
        identb = pp.tile([P, P], BF16, tag="identb", name="identb")
        nc.vector.tensor_copy(identb, ident)
        # Dummy exp: loads the ACT Exp table during the prologue.
        warm = pp.tile([P, 1], F32, tag="warm", name="warm")
        nc.scalar.activation(warm, ident[:, 0:1], AF.Exp)
        # zero rhs for explicit PSUM-bank zeroing matmuls
        zb = pp.tile([P, 512], BF16, tag="zb", name="zb")
        nc.vector.memset(zb, 0.0)

        # ---------- DMA prologue (order == service order; DMA dev ~serial)
        # Critical chain: X-c0 -> wqkv -> X-c1 feed the warmup sweep; b0's
        # c2/c3 go via the bf16-DRAM roundtrip + DMA transpose (no PE cost),
        # landing just in time for the half1 sweep.
        wqkv_sb = pp.tile([P, DCH, 384], BF16, tag="wqkv", name="wqkv_sb")
        wq_sb = wqkv_sb[:, :, 0:128]
        wk_sb = wqkv_sb[:, :, 128:256]
        wv_sb = wqkv_sb[:, :, 256:384]
        xb0 = []
        for c in range(NCH):
            t = pp.tile([P, 4, D], BF16, tag="xb0", bufs=NCH, name=f"xb0_{c}")
            src = X[0, c * 512 : (c + 1) * 512, :].rearrange(
                "(ns p) d -> p ns d", p=P
            )
            nc.gpsimd.dma_start(t, src)
            if c == 0:
                nc.gpsimd.dma_start(
                    wqkv_sb, WQKV.rearrange("(dc p) m -> p dc m", p=P)
                )
            xb0.append(t)
        # W_O natural rows: wo_sb[:, ic, :] holds rows [128*ic, 128*(ic+1))
        wo_sb = pp.tile([P, DCH, DOUT], BF16, tag="wo", name="wo_sb")
        xt1 = []
        for c in range(NCH):
            t = pp.tile([P, DCH, 512], BF16, tag="xt1", bufs=NCH, name=f"xt1_{c}")
            xt1.append(t)

        def emit_late_dmas():
            """The framework's (whole-tensor) XB dependency tracking orders
            these casts after the lead-critical xt0 DMA-transposes; wo goes
            last so its 5.8us never splits the xt0 chain on the serialized
            DMA device."""
            for c in range(NCH):
                ci = nc.gpsimd.dma_start(
                    XB[1, c * 512 : (c + 1) * 512, :],
                    X[1, c * 512 : (c + 1) * 512, :],
                )
                tc.chain_iter_dep("dmaorder", ci.ins)
                ti = nc.sync.dma_start_transpose(
                    xt1[c], XB[1, c * 512 : (c + 1) * 512, :]
                )
                tc.chain_iter_dep("dmaorder", ti.ins)
            wi = nc.gpsimd.dma_start(wo_sb, WO.rearrange("(ic p) o -> p ic o", p=P))
            tc.chain_iter_dep("dmaorder", wi.ins)

        qT, kT, v_sb = [], [], []
        for b in range(B):
            qT.append(pp.tile([P, N], BF16, tag=f"qT{b}", name=f"qT{b}"))
            kT.append(pp.tile([P, N], BF16, tag=f"kT{b}", name=f"kT{b}"))
            v_sb.append(pp.tile([P, JKB, P], BF16, tag=f"v{b}", name=f"v{b}"))

        # xtc: b0's X^T chunks (PE-transposed), kept for K/V(b0) pump units.
        # Registered as per-(c, dc) APs (lead uses 4-dc tiles, the warmup
        # 2-dc tiles in aux slots).
        xtc_ap = {}  # (c, dc) -> [P, 512] bf16 AP

        def xtc(c, dc):
            return xtc_ap[(c, dc)]

        # avt PSUM accumulators AV^T[h]: [i-queries, (ib, dv)] - 2 banks
        # aux PSUM: proj accumulators / W_O op tiles - 2x1 banks

        def emit_qk_proj(b, which, c, xsrc, width=512):
            """Projection chunk: qT/kT[b][:, c*width + [0,width)] over 8 dc.
            width=256 halves the PE burst for pump units."""
            w = wq_sb if which == "q" else wk_sb
            dst = qT[b] if which == "q" else kT[b]
            ps = aps.tile([P, 512], F32, tag="aux", bufs=2, name=f"p{which}{b}{c}")
            pw = ps[:, 0:width]
            for dc in range(DCH):
                nc.tensor.matmul(
                    pw, w[:, dc, :], xsrc(dc)[:, 0:width],
                    start=(dc == 0), stop=(dc == DCH - 1),
                )
            nc.vector.tensor_copy(dst[:, c * width : c * width + width], pw)

        def emit_v_proj(b, c2, xsrc):
            """V natural rows for TWO key-blocks (quarter-chunk c2): out
            [n-128-block, 128 dv] with x^T block as stationary. The range is
            zeroed with one wide matmul first so the two column-disjoint
            accumulation groups can use start=False (PSUM lazy-zero marks
            whole regions; per-group start=True would clobber siblings)."""
            ps = aps.tile([P, 512], F32, tag="aux", bufs=2, name=f"pv{b}{c2}")
            pw = ps[:, 0:256]
            nc.tensor.matmul(pw, identb, zb[:, 0:256], start=True, stop=False,
                             skip_group_check=True)
            for dc in range(DCH):
                for jb in range(2):
                    nc.tensor.matmul(
                        pw[:, jb * P : (jb + 1) * P],
                        xsrc(dc)[:, (c2 % 2) * 256 + jb * P : (c2 % 2) * 256 + (jb + 1) * P],
                        wv_sb[:, dc, :],
                        start=False, stop=(dc == DCH - 1),
                        skip_group_check=True,
                    )
            nc.vector.tensor_copy(
                v_sb[b][:, c2 * 2 : (c2 + 1) * 2, :], pw
            )

        # ---------------- lead-in: b0 c0/c1 transposes + Q + K(c0,c1)
        # tp borrows "st" slots (4KB, idle until the first window).
        for c in range(2):
            xns = [xb0[c][:, ns, :] for ns in range(4)]
            for q in range(2):
                tp = aps.tile([P, 4, 512], BF16, tag="st", bufs=2, name="tp")
                for dl in range(4):
                    dc = q * 4 + dl
                    for ns in range(4):
                        nc.tensor.transpose(
                            tp[:, dl, ns * 128 : (ns + 1) * 128],
                            xns[ns][:, dc * 128 : (dc + 1) * 128],
                            identb,
                        )
                xc = pp.tile([P, 4, 512], BF16, tag="xtc", bufs=2 * NCH,
                             name=f"xtc{c}_{q}")
                nc.vector.tensor_copy(xc, tp)
                for dl in range(4):
                    xtc_ap[(c, q * 4 + dl)] = xc[:, dl, :]
            emit_qk_proj(0, "q", c, lambda dc, c=c: xtc(c, dc))
            emit_qk_proj(0, "k", c, lambda dc, c=c: xtc(c, dc))

        # ---------------- pump units (interleaved into jk loops)
        # Order matters: a unit writing a tile must be EMITTED before any
        # reader (the tile framework only syncs backward).  V(b0)c0 first
        # (AV(jk0) reads it at jk1), then K/V alternating so kT chunk c is
        # emitted before scores jk>=4c and v chunk c before AV jk>=4c.
        # all units are ~0.9us of PE: 256-wide proj columns / 2 V-blocks.
        # xsrcq(b, c2) yields the 256-wide moving slice for quarter c2.
        def xs0(c2):
            return lambda dc, c2=c2: xtc(c2 // 2, dc)[:, (c2 % 2) * 256 : (c2 % 2 + 1) * 256]

        def xs1(c2):
            return lambda dc, c2=c2: xt1[c2 // 2][:, dc, (c2 % 2) * 256 : (c2 % 2 + 1) * 256]

        def xv0(c2):
            return lambda dc, c2=c2: xtc(c2 // 2, dc)

        def xv1(c2):
            return lambda dc, c2=c2: xt1[c2 // 2][:, dc, :]

        # (deadline_pos, fn): deadline = last pump position (w*16+jk) at
        # which the unit must be EMITTED so every reader is emitted later.
        units = []
        for c2 in range(2 * NCH):
            # reader: AV(jk=2*c2) emitted at pos 2*c2+2
            units.append((2 * c2 + 1,
                          lambda c2=c2: emit_v_proj(0, c2, xv0(c2))))
        for c2 in range(4, 2 * NCH):
            # reader: sc(jk=2*c2) at pos 2*c2
            units.append((2 * c2 - 2,
                          lambda c2=c2: emit_qk_proj(0, "k", c2, xs0(c2), 256)))
        for c2 in range(2 * NCH):
            # reader: sc(W3, jk0) at pos 32
            units.append((14 + c2,
                          lambda c2=c2: emit_qk_proj(1, "q", c2, xs1(c2), 256)))
        kdl = [22, 24, 26, 28, 32, 34, 36, 38]
        for c2 in range(2 * NCH):
            # reader: sc(W3, jk=2*c2) at pos 32+2*c2
            units.append((kdl[c2],
                          lambda c2=c2: emit_qk_proj(1, "k", c2, xs1(c2), 256)))
        for c2 in range(2 * NCH):
            # reader: AV(W3, jk=2*c2) at pos 34+2*c2
            units.append((33 + 2 * c2,
                          lambda c2=c2: emit_v_proj(1, c2, xv1(c2))))
        units.sort(key=lambda t: t[0])
        front = []       # drain/WO pieces, pumped in the slack
        pos_ref = [0]    # current pump position

        def force_due():
            # Emission-order correctness: every unit whose reader gets
            # emitted this iteration MUST be emitted first.
            while units and units[0][0] <= pos_ref[0]:
                units.pop(0)[1]()

        def pump(n=1):
            # Non-greedy: proj units only near their deadline, drain/W_O
            # pieces fill the slack, otherwise leave the PE light -- the
            # global PE budget fits but bunching starves the exp stream.
            for _ in range(n):
                if units and units[0][0] <= pos_ref[0] + 3:
                    units.pop(0)[1]()
                elif front and pos_ref[0] % JKB >= 2:
                    front.pop(0)()

        # ---------------- W_O chunk for a finished head
        def emit_wo_chunk(b, h, act, dh):
            opf = aps.tile([P, 512], F32, tag="aux", bufs=2, name="opf")
            op = opf[:R, :OC]
            for ic in range(DCH):
                nc.tensor.matmul(
                    op, act[:, ic, :],
                    wo_sb[:, ic, dh * OC : (dh + 1) * OC],
                    start=(ic == 0), stop=(ic == DCH - 1),
                )
            o_t = pp.tile([R, OC], F32, tag="ot", bufs=3, name="o_t")
            nc.vector.tensor_copy(o_t, op)
            nc.sync.dma_start(
                O[b, h * R : (h + 1) * R, dh * OC : (dh + 1) * OC], o_t
            )

        def emit_drain_pieces(b, h, a_sb, via_front):
            """PE-transpose avts blocks to AV^T-natural, assemble AVc^T
            ([s*64+dv partitions, r]) chunks, then the 4 W_O chunks.
            The transpose targets borrow aux slots (two 2KB halves)."""
            act = pp.tile([P, DCH, P], BF16, tag="avct", bufs=2, name="avct")

            def transposes(half):
                ps = aps.tile([P, HF], BF16, tag="aux", bufs=2, name="psT")
                for cb in range(8):
                    ib = half * 8 + cb
                    nc.tensor.transpose(
                        ps[0:64, cb * 128 : (cb + 1) * 128],
                        a_sb[:, ib * 64 : (ib + 1) * 64],
                        identb,
                    )
                psr = ps[0:64].rearrange(
                    "p (r s2 par) -> p par s2 r", par=2, s2=8
                )
                rs = slice(half * 64, (half + 1) * 64)
                nc.vector.tensor_copy(act[0:64, :, rs], psr[:, 0])
                nc.vector.tensor_copy(act[64:128, :, rs], psr[:, 1])

            def wo_pair(d0):
                emit_wo_chunk(b, h, act, d0)
                emit_wo_chunk(b, h, act, d0 + 1)

            pieces = [lambda: transposes(0), lambda: transposes(1),
                      lambda: wo_pair(0), lambda: wo_pair(2)]
            if via_front:
                front.extend(pieces)
            else:
                for p in pieces:
                    p()

        # ---------------- attention: sequential heads, pipelined jk loop.
        # AV(jk) is emitted TWO jk steps late: its dependency chain
        # (exp accum -> DVE nsum/recip/scale -> 16 matmuls) resolves ~1.5us
        # after exp(jk) ends; at lag 1 it head-of-line-blocks the next
        # scores in the in-order PE queue and stretches every jk period.
        pending_av = []  # (b, hs, avt, jk, es0, es1, nsum)

        def emit_av(b, hs, avt, jk, es0, es1, nsum):
            n1 = pp.tile([P, 1], F32, tag="n1", bufs=6, name="n1")
            nc.vector.reduce_sum(n1, nsum, axis=mybir.AxisListType.X)
            nr = pp.tile([P, 1], F32, tag="nr", bufs=6, name="nr")
            nc.vector.reciprocal(nr, n1)
            vs = pp.tile([P, 64], BF16, tag="vs", bufs=6, name="vs")
            nc.vector.tensor_scalar_mul(vs, v_sb[b][:, jk, hs], nr)
            for half, es in ((0, es0), (1, es1)):
                for cb in range(HF // 128):
                    ib = half * 8 + cb
                    nc.tensor.matmul(
                        avt[:, ib * 64 : (ib + 1) * 64],
                        es[:, cb * 128 : (cb + 1) * 128],
                        vs,
                        start=False, stop=(jk == JKB - 1),
                        skip_group_check=True,
                    )

        def pop_av(n=1, min_len=2):
            # Only emit entries that are >= 2 jk steps old (lag-2 pipeline);
            # the tail flushes with min_len=1.
            for _ in range(n):
                if len(pending_av) >= min_len:
                    emit_av(*pending_av.pop(0))

        def emit_half(b, hs, jk, half, nsum, dve_sum=False):
            st = aps.tile([P, HF], F32, tag="st", bufs=2, name="st")
            i0 = half * HF
            for cq in range(2):
                nc.tensor.matmul(
                    st[:, cq * 512 : (cq + 1) * 512],
                    kT[b][hs, jk * 128 : (jk + 1) * 128],
                    qT[b][hs, i0 + cq * 512 : i0 + (cq + 1) * 512],
                    start=True, stop=True,
                )
            e = pp.tile([P, HF], BF16, tag="e", bufs=14, name="e")
            if dve_sum:
                # ACT-bound windows: skip the 187ns accumulator-read aux op;
                # DVE (which has slack there) sums the tile instead.
                nc.scalar.activation(e, st, AF.Exp)
                nc.vector.reduce_sum(
                    nsum[:, half : half + 1], e, axis=mybir.AxisListType.X
                )
            else:
                nc.scalar.activation(
                    e, st, AF.Exp, accum_out=nsum[:, half : half + 1]
                )
            return e

        def attention_window(b, h, prev, widx):
            hs = slice(h * 64, (h + 1) * 64)
            avt = None
            jk_start = 0
            if widx == 0:
                # Warmup: half0 of jk0-7 only needs qT chunks 0-1 and kT
                # chunks 0-1 (lead-in) -- exp starts ~15us while the c2/c3
                # DMA-transposes + Q projections finish underneath.
                warm_es0, warm_nsum = [], []
                for jk in range(7):
                    nsum = pp.tile([P, 2], F32, tag="nsum", bufs=10, name="nsum")
                    warm_nsum.append(nsum)
                    warm_es0.append(emit_half(b, hs, jk, 0, nsum))
                for c in (2, 3):
                    xns = [xb0[c][:, ns, :] for ns in range(4)]
                    for pr in range(4):
                        tp = aps.tile([P, 2, 512], BF16, tag="aux", bufs=2,
                                      name="tpw")
                        for dl in range(2):
                            dc = pr * 2 + dl
                            for ns in range(4):
                                nc.tensor.transpose(
                                    tp[:, dl, ns * 128 : (ns + 1) * 128],
                                    xns[ns][:, dc * 128 : (dc + 1) * 128],
                                    identb,
                                )
                        xc = pp.tile([P, 2, 512], BF16, tag="xtcp",
                                     bufs=8, name=f"xtc{c}_{pr}")
                        nc.vector.tensor_copy(xc, tp)
                        for dl in range(2):
                            xtc_ap[(c, pr * 2 + dl)] = xc[:, dl, :]
                    emit_qk_proj(0, "q", c, lambda dc, c=c: xtc(c, dc))
                emit_late_dmas()
                avt = aps.tile([P, JKB * 64], F32, tag="avt", bufs=1,
                               name="avt00")
                for bank in range(JKB * 64 // 512):
                    nc.tensor.matmul(
                        avt[:, bank * 512 : (bank + 1) * 512],
                        identb, zb, start=True, stop=False,
                        skip_group_check=True,
                    )
                for jk in range(7):
                    pos_ref[0] = jk
                    e1 = emit_half(b, hs, jk, 1, warm_nsum[jk])
                    force_due()
                    pop_av(1)
                    pump(1)
                    pending_av.append(
                        (b, hs, avt, jk, warm_es0[jk], e1, warm_nsum[jk])
                    )
                jk_start = 7
            for jk in range(jk_start, JKB):
                pos_ref[0] = widx * JKB + jk
                es = []
                nsum = pp.tile([P, 2], F32, tag="nsum", bufs=10, name="nsum")
                for half in range(2):
                    es.append(emit_half(b, hs, jk, half, nsum,
                                         dve_sum=False))
                if jk == 2:
                    # The previous window's last pending AV was emitted at
                    # jk1; its avt is final: drain it, free the slot,
                    # allocate + zero ours (before popping this window's
                    # own jk0 entry), queue its layout/W_O pieces.
                    if prev is not None:
                        pb, ph, pavt = prev
                        a_sb = pp.tile([P, JKB * 64], BF16, tag="avts",
                                       bufs=2, name="avts")
                        nc.vector.tensor_copy(a_sb, pavt)
                    avt = aps.tile([P, JKB * 64], F32, tag="avt", bufs=1,
                                   name=f"avt{b}{h}")
                    for bank in range(JKB * 64 // 512):
                        nc.tensor.matmul(
                            avt[:, bank * 512 : (bank + 1) * 512],
                            identb, zb, start=True, stop=False,
                            skip_group_check=True,
                        )
                    for i, ent in enumerate(pending_av):
                        if ent[2] is None:
                            pending_av[i] = ent[:2] + (avt,) + ent[3:]
                    force_due()
                    pop_av(1)
                    if prev is not None:
                        emit_drain_pieces(pb, ph, a_sb, via_front=True)
                else:
                    force_due()
                    pop_av(1)
                if widx == 3 and jk >= JKB - 2:
                    # last window: drain the AV backlog before the tail
                    pop_av(1, min_len=1)
                pump(1)
                pending_av.append((b, hs, avt, jk, es[0], es[1], nsum))
            return avt

        prev = None  # (b, h, avt)
        for b in range(B):
            for h in range(HL):
                avt = attention_window(b, h, prev, b * HL + h)
                prev = (b, h, avt)
        # tail: only AV(15) remains; half-split drain interleaves DVE/PE
        pop_av(2, min_len=1)
        pb, ph, pavt = prev
        a0 = pp.tile([P, 512], BF16, tag="avtt", bufs=2, name="a0")
        a1 = pp.tile([P, 512], BF16, tag="avtt", bufs=2, name="a1")
        nc.vector.tensor_copy(a0, pavt[:, 0:512])
        nc.vector.tensor_copy(a1, pavt[:, 512:1024])
        act = pp.tile([P, DCH, P], BF16, tag="avct", bufs=2, name="avct_t")
        for half, asrc in ((0, a0), (1, a1)):
            ps = aps.tile([P, HF], BF16, tag="aux", bufs=2, name="psT_t")
            for cb in range(8):
                nc.tensor.transpose(
                    ps[0:64, cb * 128 : (cb + 1) * 128],
                    asrc[:, cb * 64 : (cb + 1) * 64],
                    identb,
                )
            psr = ps[0:64].rearrange("p (r s2 par) -> p par s2 r", par=2, s2=8)
            rs = slice(half * 64, (half + 1) * 64)
            nc.vector.tensor_copy(act[0:64, :, rs], psr[:, 0])
            nc.vector.tensor_copy(act[64:128, :, rs], psr[:, 1])
        for dh in range(DOUT // OC):
            emit_wo_chunk(pb, ph, act, dh)


def build_nc(N=2048, D=1024, DOUT=1024, enable_asserts=False):
    """Build and compile the per-core Bass module. Returns nc."""
    nc = bacc.Bacc(
        "TRN2",
        target_bir_lowering=False,
        debug=False,
        enable_asserts=enable_asserts,
    )
    R = N // 16
    X = nc.dram_tensor("X", [2, N, D], F32R, kind="ExternalInput").ap()
    WQKV = nc.dram_tensor("WQKV", [D, 384], F32R, kind="ExternalInput").ap()
    WO = nc.dram_tensor("WO", [16 * 64, DOUT], F32, kind="ExternalInput").ap()
    XB = nc.dram_tensor("XB", [2, N, D], BF16, kind="Internal").ap()
    O = nc.dram_tensor("O", [2, 2 * R, DOUT], F32, kind="ExternalOutput").ap()
    with tile.TileContext(nc) as tc:
        build_attn(tc, X, WQKV, WO, XB, O, N, D, DOUT)
    nc.compile()
    return nc


_NC_CACHE = {}


def _get_nc():
    if "full" not in _NC_CACHE:
        _NC_CACHE["full"] = build_nc()
    return _NC_CACHE["full"]


class _PjrtRunner:
    """Cached jitted SPMD executor (mirrors bass2jax.run_bass_via_pjrt but
    keeps the jitted callable so repeat calls skip re-trace/re-compile)."""

    def __init__(self, nc, n_cores=8):
        import jax
        from jax.experimental.shard_map import shard_map
        from jax.sharding import Mesh, PartitionSpec
        from concourse import bass2jax

        bass2jax.install_neuronx_cc_hook()
        self.n_cores = n_cores
        partition_name = (
            nc.partition_id_tensor.name if nc.partition_id_tensor else None
        )
        in_names, out_names, out_avals, zero_outs = [], [], [], []
        for alloc in nc.m.functions[0].allocations:
            if not isinstance(alloc, mybir.MemoryLocationSet):
                continue
            name = alloc.memorylocations[0].name
            if alloc.kind == "ExternalInput":
                if name != partition_name:
                    in_names.append(name)
            elif alloc.kind == "ExternalOutput":
                out_names.append(name)
                shape = tuple(alloc.tensor_shape)
                dtype = mybir.dt.np(alloc.dtype)
                out_avals.append(jax.core.ShapedArray(shape, dtype))
                zero_outs.append(np.zeros(shape, dtype))
        self.in_names = in_names
        self.out_names = out_names
        self.out_avals = out_avals
        self.zero_outs = zero_outs
        n_params = len(in_names)
        n_outs = len(out_names)
        all_names = list(in_names + out_names)
        if partition_name is not None:
            all_names.append(partition_name)
        all_names = tuple(all_names)

        def _body(*args):
            operands = list(args)
            if partition_name is not None:
                operands.append(bass2jax.partition_id_tensor())
            outs = bass2jax._bass_exec_p.bind(
                *operands,
                out_avals=tuple(out_avals),
                in_names=all_names,
                out_names=tuple(out_names),
                lowering_input_output_aliases=(),
                sim_require_finite=True,
                sim_require_nnan=True,
                nc=nc,
            )
            return tuple(outs)

        devices = jax.devices()[:n_cores]
        mesh = Mesh(np.asarray(devices), ("core",))
        donate = tuple(range(n_params, n_params + n_outs))
        self._fn = jax.jit(
            shard_map(
                _body,
                mesh=mesh,
                in_specs=(PartitionSpec("core"),) * (n_params + n_outs),
                out_specs=(PartitionSpec("core"),) * n_outs,
                check_rep=False,
            ),
            donate_argnums=donate,
            keep_unused=True,
        )

    def __call__(self, in_maps):
        import jax

        n = self.n_cores
        concat_in = [
            np.concatenate([np.asarray(m[nm]) for m in in_maps], axis=0)
            for nm in self.in_names
        ]
        concat_zeros = [
            np.zeros((n * z.shape[0], *z.shape[1:]), z.dtype) for z in self.zero_outs
        ]
        outs = self._fn(*concat_in, *concat_zeros)
        outs = [np.asarray(o) for o in jax.block_until_ready(outs)]
        return [
            {
                nm: outs[i].reshape(n, *self.out_avals[i].shape)[c]
                for i, nm in enumerate(self.out_names)
            }
            for c in range(n)
        ]


def _get_runner():
    if "runner" not in _NC_CACHE:
        _NC_CACHE["runner"] = _PjrtRunner(_get_nc())
    return _NC_CACHE["runner"]


def _make_in_maps(X, W_Q, W_K, W_V, W_O):
    X = np.ascontiguousarray(np.asarray(X), dtype=np.float32)
    W_Q = np.asarray(W_Q, dtype=np.float32)
    W_K = np.asarray(W_K, dtype=np.float32)
    W_V = np.asarray(W_V, dtype=np.float32)
    W_O = np.ascontiguousarray(np.asarray(W_O), dtype=np.float32)
    in_maps = []
    for c in range(8):
        wqkv = np.ascontiguousarray(
            np.concatenate(
                [W_Q[2 * c], W_Q[2 * c + 1], W_K[2 * c], W_K[2 * c + 1],
                 W_V[2 * c], W_V[2 * c + 1]], axis=1
            ),
            dtype=np.float32,
        )
        in_maps.append({"X": X, "WQKV": wqkv, "WO": W_O})
    return in_maps


def kernel_with_results(X, W_Q, W_K, W_V, W_O, **run_kwargs):
    """Run via run_bass_kernel_spmd (supports trace kwargs); returns results."""
    nc = _get_nc()
    in_maps = _make_in_maps(X, W_Q, W_K, W_V, W_O)
    res = run_bass_kernel_spmd(nc, in_maps, core_ids=list(range(8)), **run_kwargs)
    return np.concatenate([r["O"] for r in res.results], axis=1), res


def kernel(X, W_Q, W_K, W_V, W_O):
    """Full-input entry point. X [2,2048,1024], W_Q/K/V [16,1024,64],
    W_O [1024,1024] -> [2,2048,1024] fp32."""
    try:
        runner = _get_runner()
        results = runner(_make_in_maps(X, W_Q, W_K, W_V, W_O))
        return np.concatenate([r["O"] for r in results], axis=1)
    except Exception:
        out, _ = kernel_with_results(X, W_Q, W_K, W_V, W_O)
        return out
